# revision 31
# baseline (speedup 1.0000x reference)
"""Trainium2 Bass kernel for nn_AttentionBlock (B=16, C=512, H=W=32, 8 heads, d_k=64).

Sharding: data-parallel over batch; each of the 8 NeuronCores computes 2 batches.

All matmuls fp16 operands (1 cycle/row + fast weight load), fp32 PSUM accumulate.
Layout is fully transposed (channels on partitions) so no transposes are needed:
  qkT projection  : qpair[p] = [q_{2p}; q_{2p+1}]^T; kpad = k zero-padded to 128 rows
  v projection    : v_aug[:, t, h, 0:64] = v tokens, [..., 64] = 1.0
  attention       : scoresT = kpad.T @ qpair; expT = exp(scale*s - 4.5) (ACT -> fp16;
                    the constant shift keeps exp inside fp16 range, softmax-invariant)
                    [res; sumexp] = [v|1].T @ expT   (M=65 matmul, accum over j tiles)
                    res_norm = res * recip(sumexp)   (DVE + DMA broadcast via DRAM)
  out projection  : outT = W_out.T-chunks @ res_norm + (x + b_out)  (host-prefused)

The attention phase is ACT(exp)-bound with PE slack, so the next batch's
projection matmuls and the previous batch's output-projection matmuls are
interleaved into the attention emission order as filler work for the PE.
"""
from collections import deque

import numpy as np

import concourse.bass as bass
from concourse import bacc
import concourse.mybir as mybir
import concourse.tile as tile
from concourse import bass_utils

F32 = mybir.dt.float32
F16 = mybir.dt.float16
AF = mybir.ActivationFunctionType
ALU = mybir.AluOpType

N_HEADS = 8
DK = 64
SCALE = DK ** -0.5
EXP_SHIFT = -4.5
C = 512
N = 1024            # tokens per batch (32*32)
NB = 2              # batches per core
NCORES = 8
NCH = C // 128      # 4 contraction chunks
NT = N // 128       # 8 token tiles
NPAIR = N_HEADS // 2


def build():
    nc = bacc.Bacc(None, target_bir_lowering=False, num_swdge_queues=4)
    x_d = nc.dram_tensor("x", (NB, C, N), F32, kind="ExternalInput")
    xpb_d = nc.dram_tensor("xpb", (NB, C, N), F32, kind="ExternalInput")
    wqk_d = nc.dram_tensor("w_qk", (C, 2, NPAIR, 128), F32, kind="ExternalInput")
    bqkt_d = nc.dram_tensor("b_qk_t", (128, 2, NPAIR), F32, kind="ExternalInput")
    wv_d = nc.dram_tensor("w_v", (C, C), F32, kind="ExternalInput")
    bv_d = nc.dram_tensor("b_v", (1, C), F32, kind="ExternalInput")
    wout_d = nc.dram_tensor("w_out", (C, C), F32, kind="ExternalInput")
    y_d = nc.dram_tensor("y", (NB, C, N), F32, kind="ExternalOutput")

    with tile.TileContext(nc) as tc:
        with (
            tc.tile_pool(name="const", bufs=1) as const,
            tc.tile_pool(name="persist", bufs=1) as persist,
            tc.tile_pool(name="sbwork", bufs=3) as sbwork,
            tc.tile_pool(name="sbexp", bufs=4) as sbexp,
            tc.tile_pool(name="ps_s", bufs=2, space="PSUM") as ps_s,
            tc.tile_pool(name="ps_acc", bufs=4, space="PSUM") as ps_acc,
            tc.tile_pool(name="dram", bufs=8, space="DRAM") as dram,
        ):
            # ---- weights / constants (staged f32 -> fp16 cast) ----
            x_r = [persist.tile([128, N], F16, name=f"xr{ch}")
                   for ch in range(NCH)]

            def staged_load(dst, src_ap, eng=None):
                st = sbwork.tile(list(src_ap.shape), F32, tag="stage", bufs=3,
                                 name="stage")
                (eng or nc.sync).dma_start(st[:], src_ap)
                nc.vector.tensor_copy(dst, st[:])

            wqk = []
            for ch in range(NCH):
                w = const.tile([128, 2, NPAIR, 128], F16, name=f"wqk{ch}")
                staged_load(w[:, 0], wqk_d[ch * 128:(ch + 1) * 128, 0],
                            nc.gpsimd)
                staged_load(x_r[ch][:], x_d[0, ch * 128:(ch + 1) * 128, :])
                staged_load(w[:, 1], wqk_d[ch * 128:(ch + 1) * 128, 1],
                            nc.gpsimd)
                wqk.append(w)
            wv = []
            for ch in range(NCH):
                w2 = const.tile([128, C], F16, name=f"wv{ch}")
                staged_load(w2[:], wv_d[ch * 128:(ch + 1) * 128, :], nc.gpsimd)
                wv.append(w2)
            bqkt = const.tile([128, 2, NPAIR], F32)
            nc.sync.dma_start(bqkt[:], bqkt_d[:])
            bv_bc = const.tile([128, C], F32)   # b_v broadcast to all partitions
            nc.sync.dma_start(bv_bc[:], bv_d[:].to_broadcast([128, C]))

            ones_f = const.tile([128, 8], F32)
            nc.vector.memset(ones_f[:], 1.0)
            expbias = const.tile([128, 1], F32)
            nc.vector.memset(expbias[:], EXP_SHIFT)
            zeros_f = const.tile([64, N], F32)
            nc.vector.memset(zeros_f[:], 0.0)

            # ---- persistent per-batch buffers ----
            qpair = [persist.tile([128, N], F16, name=f"qpair{p}")
                     for p in range(NPAIR)]
            kpad = [[persist.tile([128, N], F16, name=f"kpad{p}_{s}")
                     for s in range(2)] for p in range(NPAIR)]
            for p in range(NPAIR):   # zero pad halves once; never overwritten
                nc.vector.tensor_copy(kpad[p][0][64:128, :], zeros_f[:])
                nc.vector.tensor_copy(kpad[p][1][0:64, :], zeros_f[:])
            v_aug = persist.tile([128, NT, N_HEADS, DK + 1], F16)
            nc.vector.tensor_copy(
                v_aug[:, :, :, DK:DK + 1],
                ones_f[:, 0:1].to_broadcast([128, NT, N_HEADS]).unsqueeze(3))
            res_all_db = [[persist.tile([128, N], F16, name=f"resall{bb}_{p}")
                           for p in range(NPAIR)] for bb in range(NB)]

            # ---- work units (closures) for PE-filler interleaving ----
            def xload_unit(b, ch):
                def f():
                    staged_load(x_r[ch][:], x_d[b, ch * 128:(ch + 1) * 128, :])
                return f

            def qkT_unit(p, qk, nh):
                def f():
                    nsl = slice(nh * 512, nh * 512 + 512)
                    ps = ps_acc.tile([128, 512], F32, tag="acc", name="qk_ps")
                    for ch in range(NCH):
                        nc.tensor.matmul(
                            ps[:], wqk[ch][:, qk, p, :], x_r[ch][:, nsl],
                            start=(ch == 0), stop=(ch == NCH - 1))
                    if qk == 0:
                        nc.vector.tensor_scalar(
                            out=qpair[p][:, nsl], in0=ps[:],
                            scalar1=bqkt[:, 0, p:p + 1], scalar2=None,
                            op0=ALU.add)
                    else:
                        nc.vector.tensor_scalar(
                            out=kpad[p][0][0:64, nsl], in0=ps[0:64, :],
                            scalar1=bqkt[0:64, 1, p:p + 1], scalar2=None,
                            op0=ALU.add)
                        nc.vector.tensor_scalar(
                            out=kpad[p][1][64:128, nsl], in0=ps[64:128, :],
                            scalar1=bqkt[64:128, 1, p:p + 1], scalar2=None,
                            op0=ALU.add)
                return f

            def v_unit(t):
                def f():
                    ps = ps_acc.tile([128, 512], F32, tag="acc", name="v_ps")
                    for ch in range(NCH):
                        nc.tensor.matmul(
                            ps[:], x_r[ch][:, t * 128:(t + 1) * 128], wv[ch][:],
                            start=(ch == 0), stop=(ch == NCH - 1))
                    nc.vector.tensor_add(
                        v_aug[:, t, :, 0:DK],
                        ps[:].rearrange("p (h d) -> p h d", h=N_HEADS),
                        bv_bc[:].rearrange("p (h d) -> p h d", h=N_HEADS))
                return f

            def out_units(b):
                xres = {}
                units = []

                def mk(ct, nh):
                    def f():
                        csl = slice(ct * 128, (ct + 1) * 128)
                        nsl = slice(nh * 512, nh * 512 + 512)
                        if ct not in xres:
                            xr = sbwork.tile([128, N], F32, tag="xres", bufs=4,
                                             name=f"x_res{b}_{ct}")
                            nc.sync.dma_start(xr[:], xpb_d[b, csl, :])
                            xres[ct] = xr
                        ps = ps_acc.tile([128, 512], F32, tag="acc",
                                         name="out_ps")
                        for ch in range(NCH):
                            nc.tensor.matmul(
                                ps[:], wo[ch][:, csl],
                                res_all_db[b][ch][:, nsl],
                                start=(ch == 0), stop=(ch == NCH - 1))
                        out_sb = sbwork.tile([128, 512], F32, tag="out",
                                             name="out_sb")
                        nc.vector.tensor_add(out_sb[:], ps[:],
                                             xres[ct][:, nsl])
                        nc.sync.dma_start(y_d[b, csl, nsl], out_sb[:])
                    return f

                for nh in range(2):
                    for ct in range(NCH):
                        units.append(mk(ct, nh))
                return units[:NCH], units[NCH:]

            filler = deque()

            def inject(k=1):
                for _ in range(min(k, len(filler))):
                    filler.popleft()()

            def attn_pair(b, p, ic_fillers=None, pre_fillers=None):
                pre = deque(pre_fillers or [])
                with nc.named_scope(f"b{b}_attn{p}"):
                    for ic in range(2):
                        if ic_fillers and ic in ic_fillers:
                            filler.extend(ic_fillers[ic])
                        isl = slice(ic * 512, ic * 512 + 512)
                        res_ps = [ps_acc.tile([DK + 1, 512], F32, tag="acc",
                                              name=f"res_ps{s}")
                                  for s in range(2)]
                        exps = [None] * NT
                        for t in range(NT + 1):
                            if t < NT:
                                js = slice(t * 128, (t + 1) * 128)
                                s_ps = ps_s.tile([128, N], F32, tag="big",
                                                 name="s_ps")
                                nc.tensor.matmul(
                                    s_ps[:, 0:512], kpad[p][0][:, js],
                                    qpair[p][:, isl], start=True, stop=True)
                                nc.tensor.matmul(
                                    s_ps[:, 512:1024], kpad[p][1][:, js],
                                    qpair[p][:, isl], start=True, stop=True)
                                exp_sb = sbexp.tile([128, 2, 512], F16,
                                                    tag="exp", name="exp_sb")
                                nc.scalar.activation(
                                    out=exp_sb[:], in_=s_ps[:], func=AF.Exp,
                                    bias=expbias[:], scale=SCALE)
                                exps[t] = exp_sb
                                if pre:
                                    pre.popleft()()
                            if t >= 1:
                                for s in range(2):
                                    nc.tensor.matmul(
                                        res_ps[s][:],
                                        v_aug[:, t - 1, 2 * p + s, :],
                                        exps[t - 1][:, s, :],
                                        start=(t - 1 == 0),
                                        stop=(t - 1 == NT - 1))
                                inject(1)
                        # normalize: res / sumexp via reciprocal + DRAM bcast
                        for s in range(2):
                            sum_sb = sbwork.tile([1, 512], F32, tag="sumsb",
                                                 name="sum_sb")
                            nc.vector.tensor_copy(sum_sb[:],
                                                  res_ps[s][DK:DK + 1, :])
                            rcp_sb = sbwork.tile([1, 512], F32, tag="sums",
                                                 name="rcp_sb")
                            nc.vector.reciprocal_approx_fast(
                                out=rcp_sb[:], in_=sum_sb[:])
                            rcp_dram = dram.tile([1, 512], F32, tag="sumd",
                                                 name="rcp_dram")
                            nc.gpsimd.dma_start(rcp_dram[:], rcp_sb[:])
                            mult = sbwork.tile([64, 512], F32, tag="mult",
                                               name="mult")
                            nc.gpsimd.dma_start(
                                mult[:], rcp_dram[:].to_broadcast([64, 512]))
                            nc.vector.tensor_mul(
                                res_all_db[b][p][s * 64:(s + 1) * 64, isl],
                                res_ps[s][0:DK, :], mult[:])

            # ---- emission schedule (software-pipelined across phases) ----
            wo = []
            wo_units = []
            for ch in range(NCH):
                w = const.tile([128, C], F16, name=f"wout{ch}")
                wo.append(w)

                def mk_wo(ch=ch, w=w):
                    def f():
                        staged_load(w[:], wout_d[ch * 128:(ch + 1) * 128, :])
                    return f
                wo_units.append(mk_wo())

            def qkts(p):
                return [qkT_unit(p, qk, nh) for qk in range(2) for nh in range(2)]

            # head: only the first qk pair projection, then attention starts;
            # everything else rides inside attention windows as PE filler.
            with nc.named_scope("b0_proj"):
                for u in qkts(0):
                    u()

            for p in range(NPAIR):
                pre = None
                icf = None
                if p == 0:
                    pre = [v_unit(t) for t in range(NT)]   # b0 v, one tile ahead
                    filler.extend(qkts(1))
                elif p == 1:
                    filler.extend(qkts(2))
                    filler.extend(qkts(3))
                elif p == 2:
                    for ch in range(NCH):
                        filler.append(xload_unit(1, ch))
                    filler.extend(wo_units)
                    filler.extend(qkts(0))                 # b1 from here on
                elif p == 3:
                    filler.extend(qkts(1))
                    filler.extend(qkts(2))
                    icf = {1: [v_unit(t) for t in range(NT)]}
                attn_pair(0, p, icf, pre)

            with nc.named_scope("b1_proj"):
                inject(len(filler))
                for u in qkts(3):
                    u()

            # batch 1 attention, with batch-0 out-projection as PE filler
            b0_nh0, b0_nh1 = out_units(0)
            b1_nh0, b1_nh1 = out_units(1)
            b0_all = b0_nh0 + b0_nh1
            spread = [b0_all[0:3], b0_all[3:6], b0_all[6:8], []]
            for p in range(NPAIR):
                filler.extend(spread[p])
                icf = None
                if p == NPAIR - 1:
                    # nh=0 out-chains only need the ic0 halves of res_all
                    icf = {1: b1_nh0}
                attn_pair(1, p, icf)

            with nc.named_scope("b1_out"):
                inject(len(filler))
                for u in b1_nh1:
                    u()

    nc.finalize()
    return nc


_NC = None


def _get_nc():
    global _NC
    if _NC is None:
        _NC = build()
    return _NC


def make_in_maps(x, W_qkv, b_qkv, W_out, b_out):
    x = np.ascontiguousarray(np.asarray(x, np.float32)).reshape(16, C, N)
    b_out = np.asarray(b_out, np.float32)
    xpb = np.ascontiguousarray(x + b_out[None, :, None])
    w3 = np.asarray(W_qkv, np.float32).reshape(C, N_HEADS, 3, DK)
    w_qk = np.ascontiguousarray(
        np.stack([w3[:, :, 0], w3[:, :, 1]], axis=1).reshape(C, 2, NPAIR, 128))
    w_v = np.ascontiguousarray(w3[:, :, 2].reshape(C, C))
    b3 = np.asarray(b_qkv, np.float32).reshape(N_HEADS, 3, DK)
    b_qk_t = np.ascontiguousarray(
        np.stack([b3[:, 0], b3[:, 1]], axis=0)
        .reshape(2, NPAIR, 128).transpose(2, 0, 1))
    b_v = np.ascontiguousarray(b3[:, 2].reshape(1, C))
    maps = []
    for core in range(NCORES):
        maps.append({
            "x": x[core * NB:(core + 1) * NB],
            "xpb": xpb[core * NB:(core + 1) * NB],
            "w_qk": w_qk,
            "b_qk_t": b_qk_t,
            "w_v": w_v,
            "b_v": b_v,
            "w_out": np.asarray(W_out, np.float32),
        })
    return maps


def run_on_hw(in_maps, **kwargs):
    nc = _get_nc()
    return bass_utils.run_bass_kernel_spmd(
        nc, in_maps, core_ids=list(range(NCORES)), **kwargs)


def kernel(x, W_qkv, b_qkv, W_out, b_out):
    res = run_on_hw(make_in_maps(x, W_qkv, b_qkv, W_out, b_out))
    y = np.concatenate([r["y"] for r in res.results], axis=0)  # (16, C, N)
    return y.reshape(16, C, 32, 32).astype(np.float32)


# revision 32
# speedup vs baseline: 1.0053x; 1.0053x over previous
"""Trainium2 Bass kernel for nn_AttentionBlock (B=16, C=512, H=W=32, 8 heads, d_k=64).

Sharding: data-parallel over batch; each of the 8 NeuronCores computes 2 batches.

All matmuls fp16 operands (1 cycle/row + fast weight load), fp32 PSUM accumulate.
Layout is fully transposed (channels on partitions) so no transposes are needed:
  qkT projection  : qpair[p] = [q_{2p}; q_{2p+1}]^T; kpad = k zero-padded to 128 rows
  v projection    : v_aug[:, t, h, 0:64] = v tokens, [..., 64] = 1.0
  attention       : scoresT = kpad.T @ qpair; expT = exp(scale*s - 4.5) (ACT -> fp16;
                    the constant shift keeps exp inside fp16 range, softmax-invariant)
                    [res; sumexp] = [v|1].T @ expT   (M=65 matmul, accum over j tiles)
                    res_norm = res * recip(sumexp)   (DVE + DMA broadcast via DRAM)
  out projection  : outT = W_out.T-chunks @ res_norm + (x + b_out)  (host-prefused)

The attention phase is ACT(exp)-bound with PE slack, so the next batch's
projection matmuls and the previous batch's output-projection matmuls are
interleaved into the attention emission order as filler work for the PE.
"""
from collections import deque

import numpy as np

import concourse.bass as bass
from concourse import bacc
import concourse.mybir as mybir
import concourse.tile as tile
from concourse import bass_utils

F32 = mybir.dt.float32
F16 = mybir.dt.float16
AF = mybir.ActivationFunctionType
ALU = mybir.AluOpType

N_HEADS = 8
DK = 64
SCALE = DK ** -0.5
EXP_SHIFT = -4.5
C = 512
N = 1024            # tokens per batch (32*32)
NB = 2              # batches per core
NCORES = 8
NCH = C // 128      # 4 contraction chunks
NT = N // 128       # 8 token tiles
NPAIR = N_HEADS // 2


def build():
    nc = bacc.Bacc(None, target_bir_lowering=False, num_swdge_queues=4)
    x_d = nc.dram_tensor("x", (NB, C, N), F32, kind="ExternalInput")
    xpb_d = nc.dram_tensor("xpb", (NB, C, N), F32, kind="ExternalInput")
    wqk_d = nc.dram_tensor("w_qk", (C, 2, NPAIR, 128), F32, kind="ExternalInput")
    bqkt_d = nc.dram_tensor("b_qk_t", (128, 2, NPAIR), F32, kind="ExternalInput")
    wv_d = nc.dram_tensor("w_v", (C, C), F32, kind="ExternalInput")
    bv_d = nc.dram_tensor("b_v", (1, C), F32, kind="ExternalInput")
    wout_d = nc.dram_tensor("w_out", (C, C), F32, kind="ExternalInput")
    y_d = nc.dram_tensor("y", (NB, C, N), F32, kind="ExternalOutput")

    with tile.TileContext(nc) as tc:
        with (
            tc.tile_pool(name="const", bufs=1) as const,
            tc.tile_pool(name="persist", bufs=1) as persist,
            tc.tile_pool(name="sbwork", bufs=3) as sbwork,
            tc.tile_pool(name="sbexp", bufs=4) as sbexp,
            tc.tile_pool(name="ps_s", bufs=2, space="PSUM") as ps_s,
            tc.tile_pool(name="ps_acc", bufs=4, space="PSUM") as ps_acc,
            tc.tile_pool(name="dram", bufs=8, space="DRAM") as dram,
        ):
            # ---- weights / constants (staged f32 -> fp16 cast) ----
            x_r = [persist.tile([128, N], F16, name=f"xr{ch}")
                   for ch in range(NCH)]

            def staged_load(dst, src_ap, eng=None):
                st = sbwork.tile(list(src_ap.shape), F32, tag="stage", bufs=3,
                                 name="stage")
                (eng or nc.sync).dma_start(st[:], src_ap)
                nc.vector.tensor_copy(dst, st[:])

            wqk = []
            for ch in range(NCH):
                w = const.tile([128, 2, NPAIR, 128], F16, name=f"wqk{ch}")
                staged_load(w[:, 0], wqk_d[ch * 128:(ch + 1) * 128, 0])
                staged_load(x_r[ch][:], x_d[0, ch * 128:(ch + 1) * 128, :],
                            nc.gpsimd)
                staged_load(w[:, 1], wqk_d[ch * 128:(ch + 1) * 128, 1])
                wqk.append(w)
            wv = []
            for ch in range(NCH):
                w2 = const.tile([128, C], F16, name=f"wv{ch}")
                staged_load(w2[:], wv_d[ch * 128:(ch + 1) * 128, :], nc.gpsimd)
                wv.append(w2)
            bqkt = const.tile([128, 2, NPAIR], F32)
            nc.sync.dma_start(bqkt[:], bqkt_d[:])
            bv_bc = const.tile([128, C], F32)   # b_v broadcast to all partitions
            nc.sync.dma_start(bv_bc[:], bv_d[:].to_broadcast([128, C]))

            ones_f = const.tile([128, 8], F32)
            nc.vector.memset(ones_f[:], 1.0)
            expbias = const.tile([128, 1], F32)
            nc.vector.memset(expbias[:], EXP_SHIFT)
            zeros_f = const.tile([64, N], F32)
            nc.vector.memset(zeros_f[:], 0.0)

            # ---- persistent per-batch buffers ----
            qpair = [persist.tile([128, N], F16, name=f"qpair{p}")
                     for p in range(NPAIR)]
            kpad = [[persist.tile([128, N], F16, name=f"kpad{p}_{s}")
                     for s in range(2)] for p in range(NPAIR)]
            for p in range(NPAIR):   # zero pad halves once; never overwritten
                nc.vector.tensor_copy(kpad[p][0][64:128, :], zeros_f[:])
                nc.vector.tensor_copy(kpad[p][1][0:64, :], zeros_f[:])
            v_aug = persist.tile([128, NT, N_HEADS, DK + 1], F16)
            nc.vector.tensor_copy(
                v_aug[:, :, :, DK:DK + 1],
                ones_f[:, 0:1].to_broadcast([128, NT, N_HEADS]).unsqueeze(3))
            res_all_db = [[persist.tile([128, N], F16, name=f"resall{bb}_{p}")
                           for p in range(NPAIR)] for bb in range(NB)]

            # ---- work units (closures) for PE-filler interleaving ----
            def xload_unit(b, ch):
                def f():
                    staged_load(x_r[ch][:], x_d[b, ch * 128:(ch + 1) * 128, :])
                return f

            def qkT_unit(p, qk, nh):
                def f():
                    nsl = slice(nh * 512, nh * 512 + 512)
                    ps = ps_acc.tile([128, 512], F32, tag="acc", name="qk_ps")
                    for ch in range(NCH):
                        nc.tensor.matmul(
                            ps[:], wqk[ch][:, qk, p, :], x_r[ch][:, nsl],
                            start=(ch == 0), stop=(ch == NCH - 1))
                    if qk == 0:
                        nc.vector.tensor_scalar(
                            out=qpair[p][:, nsl], in0=ps[:],
                            scalar1=bqkt[:, 0, p:p + 1], scalar2=None,
                            op0=ALU.add)
                    else:
                        nc.vector.tensor_scalar(
                            out=kpad[p][0][0:64, nsl], in0=ps[0:64, :],
                            scalar1=bqkt[0:64, 1, p:p + 1], scalar2=None,
                            op0=ALU.add)
                        nc.vector.tensor_scalar(
                            out=kpad[p][1][64:128, nsl], in0=ps[64:128, :],
                            scalar1=bqkt[64:128, 1, p:p + 1], scalar2=None,
                            op0=ALU.add)
                return f

            def v_unit(t):
                def f():
                    ps = ps_acc.tile([128, 512], F32, tag="acc", name="v_ps")
                    for ch in range(NCH):
                        nc.tensor.matmul(
                            ps[:], x_r[ch][:, t * 128:(t + 1) * 128], wv[ch][:],
                            start=(ch == 0), stop=(ch == NCH - 1))
                    nc.vector.tensor_add(
                        v_aug[:, t, :, 0:DK],
                        ps[:].rearrange("p (h d) -> p h d", h=N_HEADS),
                        bv_bc[:].rearrange("p (h d) -> p h d", h=N_HEADS))
                return f

            def out_units(b):
                xres = {}
                units = []

                def mk(ct, nh):
                    def f():
                        csl = slice(ct * 128, (ct + 1) * 128)
                        nsl = slice(nh * 512, nh * 512 + 512)
                        if ct not in xres:
                            xr = sbwork.tile([128, N], F32, tag="xres", bufs=4,
                                             name=f"x_res{b}_{ct}")
                            nc.sync.dma_start(xr[:], xpb_d[b, csl, :])
                            xres[ct] = xr
                        ps = ps_acc.tile([128, 512], F32, tag="acc",
                                         name="out_ps")
                        for ch in range(NCH):
                            nc.tensor.matmul(
                                ps[:], wo[ch][:, csl],
                                res_all_db[b][ch][:, nsl],
                                start=(ch == 0), stop=(ch == NCH - 1))
                        out_sb = sbwork.tile([128, 512], F32, tag="out",
                                             name="out_sb")
                        nc.vector.tensor_add(out_sb[:], ps[:],
                                             xres[ct][:, nsl])
                        nc.sync.dma_start(y_d[b, csl, nsl], out_sb[:])
                    return f

                for nh in range(2):
                    for ct in range(NCH):
                        units.append(mk(ct, nh))
                return units[:NCH], units[NCH:]

            filler = deque()

            def inject(k=1):
                for _ in range(min(k, len(filler))):
                    filler.popleft()()

            def attn_pair(b, p, ic_fillers=None, pre_fillers=None):
                pre = deque(pre_fillers or [])
                with nc.named_scope(f"b{b}_attn{p}"):
                    for ic in range(2):
                        if ic_fillers and ic in ic_fillers:
                            filler.extend(ic_fillers[ic])
                        isl = slice(ic * 512, ic * 512 + 512)
                        res_ps = [ps_acc.tile([DK + 1, 512], F32, tag="acc",
                                              name=f"res_ps{s}")
                                  for s in range(2)]
                        exps = [None] * NT
                        for t in range(NT + 1):
                            if t < NT:
                                js = slice(t * 128, (t + 1) * 128)
                                s_ps = ps_s.tile([128, N], F32, tag="big",
                                                 name="s_ps")
                                nc.tensor.matmul(
                                    s_ps[:, 0:512], kpad[p][0][:, js],
                                    qpair[p][:, isl], start=True, stop=True)
                                nc.tensor.matmul(
                                    s_ps[:, 512:1024], kpad[p][1][:, js],
                                    qpair[p][:, isl], start=True, stop=True)
                                exp_sb = sbexp.tile([128, 2, 512], F16,
                                                    tag="exp", name="exp_sb")
                                nc.scalar.activation(
                                    out=exp_sb[:], in_=s_ps[:], func=AF.Exp,
                                    bias=expbias[:], scale=SCALE)
                                exps[t] = exp_sb
                                if pre:
                                    pre.popleft()()
                            if t >= 1:
                                for s in range(2):
                                    nc.tensor.matmul(
                                        res_ps[s][:],
                                        v_aug[:, t - 1, 2 * p + s, :],
                                        exps[t - 1][:, s, :],
                                        start=(t - 1 == 0),
                                        stop=(t - 1 == NT - 1))
                                inject(1)
                        # normalize: res / sumexp via reciprocal + DRAM bcast
                        for s in range(2):
                            sum_sb = sbwork.tile([1, 512], F32, tag="sumsb",
                                                 name="sum_sb")
                            nc.vector.tensor_copy(sum_sb[:],
                                                  res_ps[s][DK:DK + 1, :])
                            rcp_sb = sbwork.tile([1, 512], F32, tag="sums",
                                                 name="rcp_sb")
                            nc.vector.reciprocal_approx_fast(
                                out=rcp_sb[:], in_=sum_sb[:])
                            rcp_dram = dram.tile([1, 512], F32, tag="sumd",
                                                 name="rcp_dram")
                            nc.gpsimd.dma_start(rcp_dram[:], rcp_sb[:])
                            mult = sbwork.tile([64, 512], F32, tag="mult",
                                               name="mult")
                            nc.gpsimd.dma_start(
                                mult[:], rcp_dram[:].to_broadcast([64, 512]))
                            nc.vector.tensor_mul(
                                res_all_db[b][p][s * 64:(s + 1) * 64, isl],
                                res_ps[s][0:DK, :], mult[:])

            # ---- emission schedule (software-pipelined across phases) ----
            wo = []
            wo_units = []
            for ch in range(NCH):
                w = const.tile([128, C], F16, name=f"wout{ch}")
                wo.append(w)

                def mk_wo(ch=ch, w=w):
                    def f():
                        staged_load(w[:], wout_d[ch * 128:(ch + 1) * 128, :])
                    return f
                wo_units.append(mk_wo())

            def qkts(p):
                return [qkT_unit(p, qk, nh) for qk in range(2) for nh in range(2)]

            # head: only the first qk pair projection, then attention starts;
            # everything else rides inside attention windows as PE filler.
            with nc.named_scope("b0_proj"):
                for u in qkts(0):
                    u()

            for p in range(NPAIR):
                pre = None
                icf = None
                if p == 0:
                    pre = [v_unit(t) for t in range(NT)]   # b0 v, one tile ahead
                    filler.extend(qkts(1))
                elif p == 1:
                    filler.extend(qkts(2))
                    filler.extend(qkts(3))
                elif p == 2:
                    for ch in range(NCH):
                        filler.append(xload_unit(1, ch))
                    filler.extend(wo_units)
                    filler.extend(qkts(0))                 # b1 from here on
                elif p == 3:
                    filler.extend(qkts(1))
                    filler.extend(qkts(2))
                    icf = {1: [v_unit(t) for t in range(NT)]}
                attn_pair(0, p, icf, pre)

            with nc.named_scope("b1_proj"):
                inject(len(filler))
                for u in qkts(3):
                    u()

            # batch 1 attention, with batch-0 out-projection as PE filler
            b0_nh0, b0_nh1 = out_units(0)
            b1_nh0, b1_nh1 = out_units(1)
            b0_all = b0_nh0 + b0_nh1
            spread = [b0_all[0:3], b0_all[3:6], b0_all[6:8], []]
            for p in range(NPAIR):
                filler.extend(spread[p])
                icf = None
                if p == NPAIR - 1:
                    # nh=0 out-chains only need the ic0 halves of res_all
                    icf = {1: b1_nh0}
                attn_pair(1, p, icf)

            with nc.named_scope("b1_out"):
                inject(len(filler))
                for u in b1_nh1:
                    u()

    nc.finalize()
    return nc


_NC = None


def _get_nc():
    global _NC
    if _NC is None:
        _NC = build()
    return _NC


def make_in_maps(x, W_qkv, b_qkv, W_out, b_out):
    x = np.ascontiguousarray(np.asarray(x, np.float32)).reshape(16, C, N)
    b_out = np.asarray(b_out, np.float32)
    xpb = np.ascontiguousarray(x + b_out[None, :, None])
    w3 = np.asarray(W_qkv, np.float32).reshape(C, N_HEADS, 3, DK)
    w_qk = np.ascontiguousarray(
        np.stack([w3[:, :, 0], w3[:, :, 1]], axis=1).reshape(C, 2, NPAIR, 128))
    w_v = np.ascontiguousarray(w3[:, :, 2].reshape(C, C))
    b3 = np.asarray(b_qkv, np.float32).reshape(N_HEADS, 3, DK)
    b_qk_t = np.ascontiguousarray(
        np.stack([b3[:, 0], b3[:, 1]], axis=0)
        .reshape(2, NPAIR, 128).transpose(2, 0, 1))
    b_v = np.ascontiguousarray(b3[:, 2].reshape(1, C))
    maps = []
    for core in range(NCORES):
        maps.append({
            "x": x[core * NB:(core + 1) * NB],
            "xpb": xpb[core * NB:(core + 1) * NB],
            "w_qk": w_qk,
            "b_qk_t": b_qk_t,
            "w_v": w_v,
            "b_v": b_v,
            "w_out": np.asarray(W_out, np.float32),
        })
    return maps


def run_on_hw(in_maps, **kwargs):
    nc = _get_nc()
    return bass_utils.run_bass_kernel_spmd(
        nc, in_maps, core_ids=list(range(NCORES)), **kwargs)


def kernel(x, W_qkv, b_qkv, W_out, b_out):
    res = run_on_hw(make_in_maps(x, W_qkv, b_qkv, W_out, b_out))
    y = np.concatenate([r["y"] for r in res.results], axis=0)  # (16, C, N)
    return y.reshape(16, C, 32, 32).astype(np.float32)


# revision 33
# speedup vs baseline: 1.0627x; 1.0571x over previous
"""Trainium2 Bass kernel for nn_AttentionBlock (B=16, C=512, H=W=32, 8 heads, d_k=64).

Sharding: data-parallel over batch; each of the 8 NeuronCores computes 2 batches.

All matmuls fp16 operands (1 cycle/row + fast weight load), fp32 PSUM accumulate.
Layout is fully transposed (channels on partitions) so no transposes are needed:
  qkT projection  : qpair[p] = [q_{2p}; q_{2p+1}]^T; kpad = k zero-padded to 128 rows
  v projection    : v_aug[:, t, h, 0:64] = v tokens, [..., 64] = 1.0
  attention       : scoresT = kpad.T @ qpair; expT = exp(scale*s - 4.5) (ACT -> fp16;
                    the constant shift keeps exp inside fp16 range, softmax-invariant)
                    [res; sumexp] = [v|1].T @ expT   (M=65 matmul, accum over j tiles)
                    res_norm = res * recip(sumexp)   (DVE + DMA broadcast via DRAM)
  out projection  : outT = W_out.T-chunks @ res_norm + (x + b_out)  (host-prefused)

The attention phase is ACT(exp)-bound with PE slack, so the next batch's
projection matmuls and the previous batch's output-projection matmuls are
interleaved into the attention emission order as filler work for the PE.
"""
from collections import deque

import numpy as np

import concourse.bass as bass
from concourse import bacc
import concourse.mybir as mybir
import concourse.tile as tile
from concourse import bass_utils

F32 = mybir.dt.float32
F16 = mybir.dt.float16
AF = mybir.ActivationFunctionType
ALU = mybir.AluOpType

N_HEADS = 8
DK = 64
SCALE = DK ** -0.5
EXP_SHIFT = -4.5
C = 512
N = 1024            # tokens per batch (32*32)
NB = 2              # batches per core
NCORES = 8
NCH = C // 128      # 4 contraction chunks
NT = N // 128       # 8 token tiles
NPAIR = N_HEADS // 2


def build():
    nc = bacc.Bacc(None, target_bir_lowering=False, num_swdge_queues=4)
    x_d = nc.dram_tensor("x", (NB, C, N), F16, kind="ExternalInput")
    xpb_d = nc.dram_tensor("xpb", (NB, C, N), F32, kind="ExternalInput")
    wqk_d = nc.dram_tensor("w_qk", (C, 2, NPAIR, 128), F16, kind="ExternalInput")
    bqkt_d = nc.dram_tensor("b_qk_t", (128, 2, NPAIR), F32, kind="ExternalInput")
    wv_d = nc.dram_tensor("w_v", (C, C), F16, kind="ExternalInput")
    bv_d = nc.dram_tensor("b_v", (1, C), F32, kind="ExternalInput")
    wout_d = nc.dram_tensor("w_out", (C, C), F16, kind="ExternalInput")
    y_d = nc.dram_tensor("y", (NB, C, N), F32, kind="ExternalOutput")

    with tile.TileContext(nc) as tc:
        with (
            tc.tile_pool(name="const", bufs=1) as const,
            tc.tile_pool(name="persist", bufs=1) as persist,
            tc.tile_pool(name="sbwork", bufs=3) as sbwork,
            tc.tile_pool(name="sbexp", bufs=4) as sbexp,
            tc.tile_pool(name="ps_s", bufs=2, space="PSUM") as ps_s,
            tc.tile_pool(name="ps_acc", bufs=4, space="PSUM") as ps_acc,
            tc.tile_pool(name="dram", bufs=8, space="DRAM") as dram,
        ):
            # ---- weights / constants (staged f32 -> fp16 cast) ----
            x_r = [persist.tile([128, N], F16, name=f"xr{ch}")
                   for ch in range(NCH)]

            def staged_load(dst, src_ap, eng=None):
                st = sbwork.tile(list(src_ap.shape), F32, tag="stage", bufs=3,
                                 name="stage")
                (eng or nc.sync).dma_start(st[:], src_ap)
                nc.vector.tensor_copy(dst, st[:])

            wqk = []
            for ch in range(NCH):
                w = const.tile([128, 2, NPAIR, 128], F16, name=f"wqk{ch}")
                nc.sync.dma_start(w[:], wqk_d[ch * 128:(ch + 1) * 128])
                nc.gpsimd.dma_start(x_r[ch][:], x_d[0, ch * 128:(ch + 1) * 128, :])
                wqk.append(w)
            wv = []
            for ch in range(NCH):
                w2 = const.tile([128, C], F16, name=f"wv{ch}")
                nc.gpsimd.dma_start(w2[:], wv_d[ch * 128:(ch + 1) * 128, :])
                wv.append(w2)
            bqkt = const.tile([128, 2, NPAIR], F32)
            nc.sync.dma_start(bqkt[:], bqkt_d[:])
            bv_bc = const.tile([128, C], F32)   # b_v broadcast to all partitions
            nc.sync.dma_start(bv_bc[:], bv_d[:].to_broadcast([128, C]))

            ones_f = const.tile([128, 8], F32)
            nc.vector.memset(ones_f[:], 1.0)
            expbias = const.tile([128, 1], F32)
            nc.vector.memset(expbias[:], EXP_SHIFT)
            zeros_f = const.tile([64, N], F32)
            nc.vector.memset(zeros_f[:], 0.0)

            # ---- persistent per-batch buffers ----
            qpair = [persist.tile([128, N], F16, name=f"qpair{p}")
                     for p in range(NPAIR)]
            kpad = [[persist.tile([128, N], F16, name=f"kpad{p}_{s}")
                     for s in range(2)] for p in range(NPAIR)]
            for p in range(NPAIR):   # zero pad halves once; never overwritten
                nc.vector.tensor_copy(kpad[p][0][64:128, :], zeros_f[:])
                nc.vector.tensor_copy(kpad[p][1][0:64, :], zeros_f[:])
            v_aug = persist.tile([128, NT, N_HEADS, DK + 1], F16)
            nc.vector.tensor_copy(
                v_aug[:, :, :, DK:DK + 1],
                ones_f[:, 0:1].to_broadcast([128, NT, N_HEADS]).unsqueeze(3))
            res_all_db = [[persist.tile([128, N], F16, name=f"resall{bb}_{p}")
                           for p in range(NPAIR)] for bb in range(NB)]

            # ---- work units (closures) for PE-filler interleaving ----
            def xload_unit(b, ch):
                def f():
                    nc.gpsimd.dma_start(x_r[ch][:],
                                        x_d[b, ch * 128:(ch + 1) * 128, :])
                return f

            def qkT_unit(p, qk, nh):
                def f():
                    nsl = slice(nh * 512, nh * 512 + 512)
                    ps = ps_acc.tile([128, 512], F32, tag="acc", name="qk_ps")
                    for ch in range(NCH):
                        nc.tensor.matmul(
                            ps[:], wqk[ch][:, qk, p, :], x_r[ch][:, nsl],
                            start=(ch == 0), stop=(ch == NCH - 1))
                    if qk == 0:
                        nc.vector.tensor_scalar(
                            out=qpair[p][:, nsl], in0=ps[:],
                            scalar1=bqkt[:, 0, p:p + 1], scalar2=None,
                            op0=ALU.add)
                    else:
                        nc.vector.tensor_scalar(
                            out=kpad[p][0][0:64, nsl], in0=ps[0:64, :],
                            scalar1=bqkt[0:64, 1, p:p + 1], scalar2=None,
                            op0=ALU.add)
                        nc.vector.tensor_scalar(
                            out=kpad[p][1][64:128, nsl], in0=ps[64:128, :],
                            scalar1=bqkt[64:128, 1, p:p + 1], scalar2=None,
                            op0=ALU.add)
                return f

            def v_unit(t):
                def f():
                    ps = ps_acc.tile([128, 512], F32, tag="acc", name="v_ps")
                    for ch in range(NCH):
                        nc.tensor.matmul(
                            ps[:], x_r[ch][:, t * 128:(t + 1) * 128], wv[ch][:],
                            start=(ch == 0), stop=(ch == NCH - 1))
                    nc.vector.tensor_add(
                        v_aug[:, t, :, 0:DK],
                        ps[:].rearrange("p (h d) -> p h d", h=N_HEADS),
                        bv_bc[:].rearrange("p (h d) -> p h d", h=N_HEADS))
                return f

            def out_units(b):
                xres = {}
                units = []

                def mk(ct, nh):
                    def f():
                        csl = slice(ct * 128, (ct + 1) * 128)
                        nsl = slice(nh * 512, nh * 512 + 512)
                        if ct not in xres:
                            xr = sbwork.tile([128, N], F32, tag="xres", bufs=4,
                                             name=f"x_res{b}_{ct}")
                            nc.sync.dma_start(xr[:], xpb_d[b, csl, :])
                            xres[ct] = xr
                        ps = ps_acc.tile([128, 512], F32, tag="acc",
                                         name="out_ps")
                        for ch in range(NCH):
                            nc.tensor.matmul(
                                ps[:], wo[ch][:, csl],
                                res_all_db[b][ch][:, nsl],
                                start=(ch == 0), stop=(ch == NCH - 1))
                        out_sb = sbwork.tile([128, 512], F32, tag="out",
                                             name="out_sb")
                        nc.vector.tensor_add(out_sb[:], ps[:],
                                             xres[ct][:, nsl])
                        nc.sync.dma_start(y_d[b, csl, nsl], out_sb[:])
                    return f

                for nh in range(2):
                    for ct in range(NCH):
                        units.append(mk(ct, nh))
                return units[:NCH], units[NCH:]

            filler = deque()

            def inject(k=1):
                for _ in range(min(k, len(filler))):
                    filler.popleft()()

            def attn_pair(b, p, ic_fillers=None, pre_fillers=None):
                pre = deque(pre_fillers or [])
                with nc.named_scope(f"b{b}_attn{p}"):
                    for ic in range(2):
                        if ic_fillers and ic in ic_fillers:
                            filler.extend(ic_fillers[ic])
                        isl = slice(ic * 512, ic * 512 + 512)
                        res_ps = [ps_acc.tile([DK + 1, 512], F32, tag="acc",
                                              name=f"res_ps{s}")
                                  for s in range(2)]
                        exps = [None] * NT
                        for t in range(NT + 1):
                            if t < NT:
                                js = slice(t * 128, (t + 1) * 128)
                                s_ps = ps_s.tile([128, N], F32, tag="big",
                                                 name="s_ps")
                                nc.tensor.matmul(
                                    s_ps[:, 0:512], kpad[p][0][:, js],
                                    qpair[p][:, isl], start=True, stop=True)
                                nc.tensor.matmul(
                                    s_ps[:, 512:1024], kpad[p][1][:, js],
                                    qpair[p][:, isl], start=True, stop=True)
                                exp_sb = sbexp.tile([128, 2, 512], F16,
                                                    tag="exp", name="exp_sb")
                                nc.scalar.activation(
                                    out=exp_sb[:], in_=s_ps[:], func=AF.Exp,
                                    bias=expbias[:], scale=SCALE)
                                exps[t] = exp_sb
                                if pre:
                                    pre.popleft()()
                            if t >= 1:
                                for s in range(2):
                                    nc.tensor.matmul(
                                        res_ps[s][:],
                                        v_aug[:, t - 1, 2 * p + s, :],
                                        exps[t - 1][:, s, :],
                                        start=(t - 1 == 0),
                                        stop=(t - 1 == NT - 1))
                                inject(1)
                        # normalize: res / sumexp via reciprocal + DRAM bcast
                        for s in range(2):
                            sum_sb = sbwork.tile([1, 512], F32, tag="sumsb",
                                                 name="sum_sb")
                            nc.vector.tensor_copy(sum_sb[:],
                                                  res_ps[s][DK:DK + 1, :])
                            rcp_sb = sbwork.tile([1, 512], F32, tag="sums",
                                                 name="rcp_sb")
                            nc.vector.reciprocal_approx_fast(
                                out=rcp_sb[:], in_=sum_sb[:])
                            rcp_dram = dram.tile([1, 512], F32, tag="sumd",
                                                 name="rcp_dram")
                            nc.gpsimd.dma_start(rcp_dram[:], rcp_sb[:])
                            mult = sbwork.tile([64, 512], F32, tag="mult",
                                               name="mult")
                            nc.gpsimd.dma_start(
                                mult[:], rcp_dram[:].to_broadcast([64, 512]))
                            nc.vector.tensor_mul(
                                res_all_db[b][p][s * 64:(s + 1) * 64, isl],
                                res_ps[s][0:DK, :], mult[:])

            # ---- emission schedule (software-pipelined across phases) ----
            wo = []
            wo_units = []
            for ch in range(NCH):
                w = const.tile([128, C], F16, name=f"wout{ch}")
                wo.append(w)

                def mk_wo(ch=ch, w=w):
                    def f():
                        nc.sync.dma_start(w[:], wout_d[ch * 128:(ch + 1) * 128, :])
                    return f
                wo_units.append(mk_wo())

            def qkts(p):
                return [qkT_unit(p, qk, nh) for qk in range(2) for nh in range(2)]

            # head: only the first qk pair projection, then attention starts;
            # everything else rides inside attention windows as PE filler.
            with nc.named_scope("b0_proj"):
                for u in qkts(0):
                    u()

            for p in range(NPAIR):
                pre = None
                icf = None
                if p == 0:
                    pre = [v_unit(t) for t in range(NT)]   # b0 v, one tile ahead
                    filler.extend(qkts(1))
                elif p == 1:
                    filler.extend(qkts(2))
                    filler.extend(qkts(3))
                elif p == 2:
                    for ch in range(NCH):
                        filler.append(xload_unit(1, ch))
                    filler.extend(wo_units)
                    filler.extend(qkts(0))                 # b1 from here on
                elif p == 3:
                    filler.extend(qkts(1))
                    filler.extend(qkts(2))
                    icf = {1: [v_unit(t) for t in range(NT)]}
                attn_pair(0, p, icf, pre)

            with nc.named_scope("b1_proj"):
                inject(len(filler))
                for u in qkts(3):
                    u()

            # batch 1 attention, with batch-0 out-projection as PE filler
            b0_nh0, b0_nh1 = out_units(0)
            b1_nh0, b1_nh1 = out_units(1)
            b0_all = b0_nh0 + b0_nh1
            spread = [b0_all[0:3], b0_all[3:6], b0_all[6:8], []]
            for p in range(NPAIR):
                filler.extend(spread[p])
                icf = None
                if p == NPAIR - 1:
                    # nh=0 out-chains only need the ic0 halves of res_all
                    icf = {1: b1_nh0}
                attn_pair(1, p, icf)

            with nc.named_scope("b1_out"):
                inject(len(filler))
                for u in b1_nh1:
                    u()

    nc.finalize()
    return nc


_NC = None


def _get_nc():
    global _NC
    if _NC is None:
        _NC = build()
    return _NC


def make_in_maps(x, W_qkv, b_qkv, W_out, b_out):
    x = np.ascontiguousarray(np.asarray(x, np.float32)).reshape(16, C, N)
    b_out = np.asarray(b_out, np.float32)
    xpb = np.ascontiguousarray(x + b_out[None, :, None])
    w3 = np.asarray(W_qkv, np.float32).reshape(C, N_HEADS, 3, DK)
    w_qk = np.ascontiguousarray(
        np.stack([w3[:, :, 0], w3[:, :, 1]], axis=1).reshape(C, 2, NPAIR, 128))
    w_v = np.ascontiguousarray(w3[:, :, 2].reshape(C, C))
    b3 = np.asarray(b_qkv, np.float32).reshape(N_HEADS, 3, DK)
    b_qk_t = np.ascontiguousarray(
        np.stack([b3[:, 0], b3[:, 1]], axis=0)
        .reshape(2, NPAIR, 128).transpose(2, 0, 1))
    b_v = np.ascontiguousarray(b3[:, 2].reshape(1, C))
    maps = []
    for core in range(NCORES):
        maps.append({
            "x": x[core * NB:(core + 1) * NB].astype(np.float16),
            "xpb": xpb[core * NB:(core + 1) * NB],
            "w_qk": w_qk.astype(np.float16),
            "b_qk_t": b_qk_t,
            "w_v": w_v.astype(np.float16),
            "b_v": b_v,
            "w_out": np.asarray(W_out, np.float16),
        })
    return maps


def run_on_hw(in_maps, **kwargs):
    nc = _get_nc()
    return bass_utils.run_bass_kernel_spmd(
        nc, in_maps, core_ids=list(range(NCORES)), **kwargs)


def kernel(x, W_qkv, b_qkv, W_out, b_out):
    res = run_on_hw(make_in_maps(x, W_qkv, b_qkv, W_out, b_out))
    y = np.concatenate([r["y"] for r in res.results], axis=0)  # (16, C, N)
    return y.reshape(16, C, 32, 32).astype(np.float32)


# revision 34
# speedup vs baseline: 1.0885x; 1.0243x over previous
"""Trainium2 Bass kernel for nn_AttentionBlock (B=16, C=512, H=W=32, 8 heads, d_k=64).

Sharding: data-parallel over batch; each of the 8 NeuronCores computes 2 batches.

All matmuls fp16 operands (1 cycle/row + fast weight load), fp32 PSUM accumulate.
Layout is fully transposed (channels on partitions) so no transposes are needed:
  qkT projection  : qpair[p] = [q_{2p}; q_{2p+1}]^T; kpad = k zero-padded to 128 rows
  v projection    : v_aug[:, t, h, 0:64] = v tokens, [..., 64] = 1.0
  attention       : scoresT = kpad.T @ qpair; expT = exp(scale*s - 4.5) (ACT -> fp16;
                    the constant shift keeps exp inside fp16 range, softmax-invariant)
                    [res; sumexp] = [v|1].T @ expT   (M=65 matmul, accum over j tiles)
                    res_norm = res * recip(sumexp)   (DVE + DMA broadcast via DRAM)
  out projection  : outT = W_out.T-chunks @ res_norm + (x + b_out)  (host-prefused)

The attention phase is ACT(exp)-bound with PE slack, so the next batch's
projection matmuls and the previous batch's output-projection matmuls are
interleaved into the attention emission order as filler work for the PE.
"""
from collections import deque

import numpy as np

import concourse.bass as bass
from concourse import bacc
import concourse.mybir as mybir
import concourse.tile as tile
from concourse import bass_utils

F32 = mybir.dt.float32
F16 = mybir.dt.float16
AF = mybir.ActivationFunctionType
ALU = mybir.AluOpType

N_HEADS = 8
DK = 64
SCALE = DK ** -0.5
EXP_SHIFT = -4.5
C = 512
N = 1024            # tokens per batch (32*32)
NB = 2              # batches per core
NCORES = 8
NCH = C // 128      # 4 contraction chunks
NT = N // 128       # 8 token tiles
NPAIR = N_HEADS // 2


def build():
    nc = bacc.Bacc(None, target_bir_lowering=False, num_swdge_queues=4)
    x_d = nc.dram_tensor("x", (NB, C, N), F16, kind="ExternalInput")
    xpb_d = nc.dram_tensor("xpb", (NB, C, N), F32, kind="ExternalInput")
    wqk_d = nc.dram_tensor("w_qk", (C, 2, NPAIR, 128), F16, kind="ExternalInput")
    bqkt_d = nc.dram_tensor("b_qk_t", (128, 2, NPAIR), F32, kind="ExternalInput")
    wv_d = nc.dram_tensor("w_v", (C, C), F16, kind="ExternalInput")
    bv_d = nc.dram_tensor("b_v", (1, C), F32, kind="ExternalInput")
    wout_d = nc.dram_tensor("w_out", (C, C), F16, kind="ExternalInput")
    y_d = nc.dram_tensor("y", (NB, C, N), F32, kind="ExternalOutput")

    with tile.TileContext(nc) as tc:
        with (
            tc.tile_pool(name="const", bufs=1) as const,
            tc.tile_pool(name="persist", bufs=1) as persist,
            tc.tile_pool(name="sbwork", bufs=3) as sbwork,
            tc.tile_pool(name="sbexp", bufs=6) as sbexp,
            tc.tile_pool(name="ps_s", bufs=2, space="PSUM") as ps_s,
            tc.tile_pool(name="ps_acc", bufs=4, space="PSUM") as ps_acc,
            tc.tile_pool(name="dram", bufs=8, space="DRAM") as dram,
        ):
            # ---- weights / constants (staged f32 -> fp16 cast) ----
            x_r = [persist.tile([128, N], F16, name=f"xr{ch}")
                   for ch in range(NCH)]

            def staged_load(dst, src_ap, eng=None):
                st = sbwork.tile(list(src_ap.shape), F32, tag="stage", bufs=3,
                                 name="stage")
                (eng or nc.sync).dma_start(st[:], src_ap)
                nc.vector.tensor_copy(dst, st[:])

            wqk = []
            for ch in range(NCH):
                w = const.tile([128, 2, NPAIR, 128], F16, name=f"wqk{ch}")
                nc.sync.dma_start(w[:], wqk_d[ch * 128:(ch + 1) * 128])
                nc.gpsimd.dma_start(x_r[ch][:], x_d[0, ch * 128:(ch + 1) * 128, :])
                wqk.append(w)
            wv = []
            for ch in range(NCH):
                w2 = const.tile([128, C], F16, name=f"wv{ch}")
                nc.gpsimd.dma_start(w2[:], wv_d[ch * 128:(ch + 1) * 128, :])
                wv.append(w2)
            bqkt = const.tile([128, 2, NPAIR], F32)
            nc.sync.dma_start(bqkt[:], bqkt_d[:])
            bv_bc = const.tile([128, C], F32)   # b_v broadcast to all partitions
            nc.sync.dma_start(bv_bc[:], bv_d[:].to_broadcast([128, C]))

            ones_f = const.tile([128, 8], F32)
            nc.vector.memset(ones_f[:], 1.0)
            expbias = const.tile([128, 1], F32)
            nc.vector.memset(expbias[:], EXP_SHIFT)
            zeros_f = const.tile([64, N], F32)
            nc.vector.memset(zeros_f[:], 0.0)

            # ---- persistent per-batch buffers ----
            qpair = [persist.tile([128, N], F16, name=f"qpair{p}")
                     for p in range(NPAIR)]
            kpad = [[persist.tile([128, N], F16, name=f"kpad{p}_{s}")
                     for s in range(2)] for p in range(NPAIR)]
            for p in range(NPAIR):   # zero pad halves once; never overwritten
                nc.vector.tensor_copy(kpad[p][0][64:128, :], zeros_f[:])
                nc.vector.tensor_copy(kpad[p][1][0:64, :], zeros_f[:])
            v_aug = persist.tile([128, NT, N_HEADS, DK + 1], F16)
            nc.vector.tensor_copy(
                v_aug[:, :, :, DK:DK + 1],
                ones_f[:, 0:1].to_broadcast([128, NT, N_HEADS]).unsqueeze(3))
            res_all_db = [[persist.tile([128, N], F16, name=f"resall{bb}_{p}")
                           for p in range(NPAIR)] for bb in range(NB)]

            # ---- work units (closures) for PE-filler interleaving ----
            def xload_unit(b, ch):
                def f():
                    nc.gpsimd.dma_start(x_r[ch][:],
                                        x_d[b, ch * 128:(ch + 1) * 128, :])
                return f

            def qkT_unit(p, qk, nh):
                def f():
                    nsl = slice(nh * 512, nh * 512 + 512)
                    ps = ps_acc.tile([128, 512], F32, tag="acc", name="qk_ps")
                    for ch in range(NCH):
                        nc.tensor.matmul(
                            ps[:], wqk[ch][:, qk, p, :], x_r[ch][:, nsl],
                            start=(ch == 0), stop=(ch == NCH - 1))
                    if qk == 0:
                        nc.vector.tensor_scalar(
                            out=qpair[p][:, nsl], in0=ps[:],
                            scalar1=bqkt[:, 0, p:p + 1], scalar2=None,
                            op0=ALU.add)
                    else:
                        nc.vector.tensor_scalar(
                            out=kpad[p][0][0:64, nsl], in0=ps[0:64, :],
                            scalar1=bqkt[0:64, 1, p:p + 1], scalar2=None,
                            op0=ALU.add)
                        nc.vector.tensor_scalar(
                            out=kpad[p][1][64:128, nsl], in0=ps[64:128, :],
                            scalar1=bqkt[64:128, 1, p:p + 1], scalar2=None,
                            op0=ALU.add)
                return f

            def v_unit(t):
                def f():
                    ps = ps_acc.tile([128, 512], F32, tag="acc", name="v_ps")
                    for ch in range(NCH):
                        nc.tensor.matmul(
                            ps[:], x_r[ch][:, t * 128:(t + 1) * 128], wv[ch][:],
                            start=(ch == 0), stop=(ch == NCH - 1))
                    nc.vector.tensor_add(
                        v_aug[:, t, :, 0:DK],
                        ps[:].rearrange("p (h d) -> p h d", h=N_HEADS),
                        bv_bc[:].rearrange("p (h d) -> p h d", h=N_HEADS))
                return f

            def out_units(b):
                xres = {}
                units = []

                def mk(ct, nh):
                    def f():
                        csl = slice(ct * 128, (ct + 1) * 128)
                        nsl = slice(nh * 512, nh * 512 + 512)
                        if ct not in xres:
                            xr = sbwork.tile([128, N], F32, tag="xres", bufs=4,
                                             name=f"x_res{b}_{ct}")
                            nc.sync.dma_start(xr[:], xpb_d[b, csl, :])
                            xres[ct] = xr
                        ps = ps_acc.tile([128, 512], F32, tag="acc",
                                         name="out_ps")
                        for ch in range(NCH):
                            nc.tensor.matmul(
                                ps[:], wo[ch][:, csl],
                                res_all_db[b][ch][:, nsl],
                                start=(ch == 0), stop=(ch == NCH - 1))
                        out_sb = sbwork.tile([128, 512], F32, tag="out",
                                             name="out_sb")
                        nc.vector.tensor_add(out_sb[:], ps[:],
                                             xres[ct][:, nsl])
                        nc.sync.dma_start(y_d[b, csl, nsl], out_sb[:])
                    return f

                for nh in range(2):
                    for ct in range(NCH):
                        units.append(mk(ct, nh))
                return units[:NCH], units[NCH:]

            filler = deque()

            def inject(k=1):
                for _ in range(min(k, len(filler))):
                    filler.popleft()()

            def attn_pair(b, p, ic_fillers=None, pre_fillers=None):
                pre = deque(pre_fillers or [])
                with nc.named_scope(f"b{b}_attn{p}"):
                    for ic in range(2):
                        if ic_fillers and ic in ic_fillers:
                            filler.extend(ic_fillers[ic])
                        isl = slice(ic * 512, ic * 512 + 512)
                        res_ps = [ps_acc.tile([DK + 1, 512], F32, tag="acc",
                                              name=f"res_ps{s}")
                                  for s in range(2)]
                        exps = [None] * NT
                        for t in range(NT + 1):
                            if t < NT:
                                js = slice(t * 128, (t + 1) * 128)
                                s_ps = ps_s.tile([128, N], F32, tag="big",
                                                 name="s_ps")
                                nc.tensor.matmul(
                                    s_ps[:, 0:512], kpad[p][0][:, js],
                                    qpair[p][:, isl], start=True, stop=True)
                                nc.tensor.matmul(
                                    s_ps[:, 512:1024], kpad[p][1][:, js],
                                    qpair[p][:, isl], start=True, stop=True)
                                exp_sb = sbexp.tile([128, 2, 512], F16,
                                                    tag="exp", name="exp_sb")
                                nc.scalar.activation(
                                    out=exp_sb[:], in_=s_ps[:], func=AF.Exp,
                                    bias=expbias[:], scale=SCALE)
                                exps[t] = exp_sb
                                if pre:
                                    pre.popleft()()
                            if t >= 1:
                                for s in range(2):
                                    nc.tensor.matmul(
                                        res_ps[s][:],
                                        v_aug[:, t - 1, 2 * p + s, :],
                                        exps[t - 1][:, s, :],
                                        start=(t - 1 == 0),
                                        stop=(t - 1 == NT - 1))
                                inject(1)
                        # normalize: res / sumexp via reciprocal + DRAM bcast
                        for s in range(2):
                            sum_sb = sbwork.tile([1, 512], F32, tag="sumsb",
                                                 name="sum_sb")
                            nc.vector.tensor_copy(sum_sb[:],
                                                  res_ps[s][DK:DK + 1, :])
                            rcp_sb = sbwork.tile([1, 512], F32, tag="sums",
                                                 name="rcp_sb")
                            nc.vector.reciprocal_approx_fast(
                                out=rcp_sb[:], in_=sum_sb[:])
                            rcp_dram = dram.tile([1, 512], F32, tag="sumd",
                                                 name="rcp_dram")
                            nc.gpsimd.dma_start(rcp_dram[:], rcp_sb[:])
                            mult = sbwork.tile([64, 512], F32, tag="mult",
                                               name="mult")
                            nc.gpsimd.dma_start(
                                mult[:], rcp_dram[:].to_broadcast([64, 512]))
                            nc.vector.tensor_mul(
                                res_all_db[b][p][s * 64:(s + 1) * 64, isl],
                                res_ps[s][0:DK, :], mult[:])

            # ---- emission schedule (software-pipelined across phases) ----
            wo = []
            wo_units = []
            for ch in range(NCH):
                w = const.tile([128, C], F16, name=f"wout{ch}")
                wo.append(w)

                def mk_wo(ch=ch, w=w):
                    def f():
                        nc.sync.dma_start(w[:], wout_d[ch * 128:(ch + 1) * 128, :])
                    return f
                wo_units.append(mk_wo())

            def qkts(p):
                return [qkT_unit(p, qk, nh) for qk in range(2) for nh in range(2)]

            # head: only the first qk pair projection, then attention starts;
            # everything else rides inside attention windows as PE filler.
            with nc.named_scope("b0_proj"):
                for u in qkts(0):
                    u()

            for p in range(NPAIR):
                pre = None
                icf = None
                if p == 0:
                    pre = [v_unit(t) for t in range(NT)]   # b0 v, one tile ahead
                    filler.extend(qkts(1))
                elif p == 1:
                    filler.extend(qkts(2))
                    filler.extend(qkts(3))
                elif p == 2:
                    for ch in range(NCH):
                        filler.append(xload_unit(1, ch))
                    filler.extend(wo_units)
                    filler.extend(qkts(0))                 # b1 from here on
                elif p == 3:
                    filler.extend(qkts(1))
                    filler.extend(qkts(2))
                    icf = {1: [v_unit(t) for t in range(NT)]}
                attn_pair(0, p, icf, pre)

            with nc.named_scope("b1_proj"):
                inject(len(filler))
                for u in qkts(3):
                    u()

            # batch 1 attention, with batch-0 out-projection as PE filler
            b0_nh0, b0_nh1 = out_units(0)
            b1_nh0, b1_nh1 = out_units(1)
            b0_all = b0_nh0 + b0_nh1
            spread = [b0_all[0:3], b0_all[3:6], b0_all[6:8], []]
            for p in range(NPAIR):
                filler.extend(spread[p])
                icf = None
                if p == NPAIR - 1:
                    # nh=0 out-chains only need the ic0 halves of res_all
                    icf = {1: b1_nh0}
                attn_pair(1, p, icf)

            with nc.named_scope("b1_out"):
                inject(len(filler))
                for u in b1_nh1:
                    u()

    nc.finalize()
    return nc


_NC = None


def _get_nc():
    global _NC
    if _NC is None:
        _NC = build()
    return _NC


def make_in_maps(x, W_qkv, b_qkv, W_out, b_out):
    x = np.ascontiguousarray(np.asarray(x, np.float32)).reshape(16, C, N)
    b_out = np.asarray(b_out, np.float32)
    xpb = np.ascontiguousarray(x + b_out[None, :, None])
    w3 = np.asarray(W_qkv, np.float32).reshape(C, N_HEADS, 3, DK)
    w_qk = np.ascontiguousarray(
        np.stack([w3[:, :, 0], w3[:, :, 1]], axis=1).reshape(C, 2, NPAIR, 128))
    w_v = np.ascontiguousarray(w3[:, :, 2].reshape(C, C))
    b3 = np.asarray(b_qkv, np.float32).reshape(N_HEADS, 3, DK)
    b_qk_t = np.ascontiguousarray(
        np.stack([b3[:, 0], b3[:, 1]], axis=0)
        .reshape(2, NPAIR, 128).transpose(2, 0, 1))
    b_v = np.ascontiguousarray(b3[:, 2].reshape(1, C))
    maps = []
    for core in range(NCORES):
        maps.append({
            "x": x[core * NB:(core + 1) * NB].astype(np.float16),
            "xpb": xpb[core * NB:(core + 1) * NB],
            "w_qk": w_qk.astype(np.float16),
            "b_qk_t": b_qk_t,
            "w_v": w_v.astype(np.float16),
            "b_v": b_v,
            "w_out": np.asarray(W_out, np.float16),
        })
    return maps


def run_on_hw(in_maps, **kwargs):
    nc = _get_nc()
    return bass_utils.run_bass_kernel_spmd(
        nc, in_maps, core_ids=list(range(NCORES)), **kwargs)


def kernel(x, W_qkv, b_qkv, W_out, b_out):
    res = run_on_hw(make_in_maps(x, W_qkv, b_qkv, W_out, b_out))
    y = np.concatenate([r["y"] for r in res.results], axis=0)  # (16, C, N)
    return y.reshape(16, C, 32, 32).astype(np.float32)
